# revision 1
# baseline (speedup 1.0000x reference)
"""Segment mean-pool (LocalPooling1D) Trainium2 Bass kernel.

x [32, 8192, 256] f32, x_pos [32, 65] sorted int32 boundaries -> y [32, 64, 256].
y[b, j] = mean(x[b, x_pos[b,j]:x_pos[b,j+1]]), empty segments -> 0.

Strategy: data-parallel over batch, 4 rows per core on 8 cores. The 0/1
segment-indicator ind[t, j] = (pos[j] <= t < pos[j+1]) for all 64 token-tiles
of a row is built in two wide DVE ops (stride-0 broadcast of pos along the
tile axis; S = (pos - p <= 128*ti) via fused scalar_tensor_tensor, then a
shifted subtract). Segment sums accumulate on the TensorEngine as
psum += ind_tile.T @ x_tile in fp32, with even/odd token-tiles packed into
separate PE column groups (concurrent sub-array matmuls) to halve the fp32
matmul wall time. Finally y = (psum_even + psum_odd) * 1/max(count, 1).
"""

import os
import sys

import numpy as np

sys.path.insert(0, "/opt/trn_rl_repo")

import concourse.bacc as bacc
import concourse.bass as bass
import concourse.tile as tile
from concourse import mybir
from concourse.bass_utils import run_bass_kernel_spmd

dt = mybir.dt
Alu = mybir.AluOpType

# Problem constants (hardcoded per harness contract).
B, T, C, P = 32, 8192, 256, 65
NSEG = P - 1
NCORES = 8
R = B // NCORES          # batch rows per core
TOK = 128                # tokens per matmul tile (K)
KTILES = T // TOK        # 64 matmul tiles per row

CFG = {
    "blk": int(os.environ.get("KB_BLK", "8")),            # token-tiles per x DMA
    "col_pack": os.environ.get("KB_COLPACK", "1") == "1", # even/odd PE col groups
    "x_bufs": int(os.environ.get("KB_XBUFS", "12")),
    "ind_bufs": int(os.environ.get("KB_INDBUFS", "2")),
    "psum_bufs": int(os.environ.get("KB_PSUMBUFS", "2")),
    "dual_dma": os.environ.get("KB_DUALDMA", "1") == "1",
}


def build_program(cfg=CFG):
    blk = cfg["blk"]
    nblk = KTILES // blk
    col_pack = cfg["col_pack"]

    nc = bacc.Bacc("TRN2", target_bir_lowering=False, debug=False)

    x_d = nc.dram_tensor("x", [R, T, C], dt.float32, kind="ExternalInput")
    pos_d = nc.dram_tensor("x_pos", [R, P], dt.int32, kind="ExternalInput")
    y_d = nc.dram_tensor("y", [R, NSEG, C], dt.float32, kind="ExternalOutput")

    with tile.TileContext(nc) as tc:
        with (
            tc.tile_pool(name="const", bufs=1) as constp,
            tc.tile_pool(name="xp", bufs=cfg["x_bufs"]) as xp,
            tc.tile_pool(name="indp", bufs=cfg["ind_bufs"]) as indp,
            tc.tile_pool(name="smallp", bufs=4) as smallp,
            tc.tile_pool(name="outp", bufs=2) as outp,
            tc.tile_pool(name="psp", bufs=cfg["psum_bufs"], space="PSUM") as psp,
        ):
            # 128*ti along the tile axis, const across partitions/segments.
            # Values <= 8064 are exact in f32, so iota directly in f32.
            tio_b = constp.tile([TOK, KTILES, P], dt.float32)
            nc.gpsimd.iota(tio_b[:], pattern=[[TOK, KTILES], [0, P]], base=0,
                           channel_multiplier=0, allow_small_or_imprecise_dtypes=True)
            # partition index p as a per-partition scalar.
            p_iota = constp.tile([TOK, 1], dt.float32)
            nc.gpsimd.iota(p_iota[:], pattern=[[1, 1]], base=0, channel_multiplier=1,
                           allow_small_or_imprecise_dtypes=True)

            for r in range(R):
                # pos row -> [1, 65] f32, broadcast to [128, 65].
                pos_row = smallp.tile([1, P], dt.int32)
                nc.gpsimd.dma_start(pos_row[:], pos_d[r : r + 1, :])
                posf_row = smallp.tile([1, P], dt.float32)
                nc.vector.tensor_copy(posf_row[:], pos_row[:])
                pos_b = smallp.tile([TOK, P], dt.float32)
                nc.gpsimd.partition_broadcast(pos_b[:], posf_row[:])

                # counts -> 1/max(cnt, 1), partition-major [64, 1].
                pos_lo = smallp.tile([NSEG, 1], dt.int32)
                pos_hi = smallp.tile([NSEG, 1], dt.int32)
                nc.gpsimd.dma_start(pos_lo[:], pos_d[r : r + 1, 0:NSEG].rearrange("one p -> p one"))
                nc.gpsimd.dma_start(pos_hi[:], pos_d[r : r + 1, 1:P].rearrange("one p -> p one"))
                cnt_f = smallp.tile([NSEG, 1], dt.float32)
                nc.vector.tensor_tensor(cnt_f[:], pos_hi[:], pos_lo[:], op=Alu.subtract)
                cntc = smallp.tile([NSEG, 1], dt.float32)
                nc.vector.tensor_scalar(cntc[:], cnt_f[:], 1.0, None, op0=Alu.max)
                recip = smallp.tile([NSEG, 1], dt.float32)
                nc.vector.reciprocal(recip[:], cntc[:])

                # S[p, ti, j] = (pos[j] <= 128*ti + p), one fused DVE op.
                S_all = indp.tile([TOK, KTILES, P], dt.float32, tag="sall")
                nc.vector.scalar_tensor_tensor(
                    S_all[:],
                    pos_b[:, None, :].broadcast_to((TOK, KTILES, P)),
                    p_iota[:],
                    tio_b[:],
                    op0=Alu.subtract,
                    op1=Alu.is_le,
                )
                # ind[p, ti, j] = S[p, ti, j] - S[p, ti, j+1]
                ind_all = indp.tile([TOK, KTILES, NSEG], dt.float32, tag="ind")
                nc.vector.tensor_tensor(
                    ind_all[:], S_all[:, :, 0:NSEG], S_all[:, :, 1:P], op=Alu.subtract
                )

                ps = psp.tile([2 * NSEG if col_pack else NSEG, C], dt.float32)
                xr = x_d[r].rearrange("(b k p) c -> b p k c", k=blk, p=TOK)
                for b in range(nblk):
                    xt = xp.tile([TOK, blk * C], dt.float32)
                    xt_v = xt[:].rearrange("p (k c) -> p k c", k=blk)
                    eng = nc.scalar if (cfg["dual_dma"] and b % 2) else nc.sync
                    eng.dma_start(xt_v, xr[b])
                    for k in range(blk):
                        ti = b * blk + k
                        rhs = xt[:, k * C : (k + 1) * C]
                        lhsT = ind_all[:, ti, :]
                        if col_pack:
                            half = ti % 2
                            nc.tensor.matmul(
                                ps[half * NSEG : (half + 1) * NSEG, :], lhsT, rhs,
                                start=(ti == half), stop=(ti == KTILES - 2 + half),
                                tile_position=(0, half * NSEG),
                                skip_group_check=True,
                            )
                        else:
                            nc.tensor.matmul(
                                ps[:], lhsT, rhs,
                                start=(ti == 0), stop=(ti == KTILES - 1),
                            )

                out_t = outp.tile([NSEG, C], dt.float32)
                if col_pack:
                    # DVE reads one PSUM operand per op: scale each half alone.
                    half_t = outp.tile([NSEG, C], dt.float32, tag="half")
                    nc.vector.tensor_scalar(
                        half_t[:], ps[NSEG : 2 * NSEG, :], recip[:], None, op0=Alu.mult
                    )
                    nc.vector.scalar_tensor_tensor(
                        out_t[:], ps[0:NSEG, :], recip[:], half_t[:],
                        op0=Alu.mult, op1=Alu.add,
                    )
                else:
                    nc.vector.tensor_scalar(out_t[:], ps[:], recip[:], None, op0=Alu.mult)
                nc.gpsimd.dma_start(y_d[r], out_t[:])

    nc.compile()
    return nc


_PROGRAM = None


def _get_program():
    global _PROGRAM
    if _PROGRAM is None:
        _PROGRAM = build_program()
    return _PROGRAM


def kernel(x, x_pos):
    x = np.ascontiguousarray(x, dtype=np.float32)
    x_pos = np.ascontiguousarray(x_pos, dtype=np.int32)
    nc = _get_program()
    in_maps = [
        {"x": x[c * R : (c + 1) * R], "x_pos": x_pos[c * R : (c + 1) * R]}
        for c in range(NCORES)
    ]
    res = run_bass_kernel_spmd(nc, in_maps, list(range(NCORES)))
    y = np.concatenate([res.results[c]["y"] for c in range(NCORES)], axis=0)
    return y.astype(np.float32)



# revision 2
# speedup vs baseline: 1.0236x; 1.0236x over previous
"""Segment mean-pool (LocalPooling1D) Trainium2 Bass kernel.

x [32, 8192, 256] f32, x_pos [32, 65] sorted int32 boundaries -> y [32, 64, 256].
y[b, j] = mean(x[b, x_pos[b,j]:x_pos[b,j+1]]), empty segments -> 0.

Strategy: data-parallel over batch, 4 rows per core on 8 cores. Token t of a
row maps to SBUF partition p = t // 64, free-slot q = t % 64, so every
partition's x data is one contiguous 64 KB HBM chunk -> large DMA descriptors
at near-peak HBM bandwidth (vs 1 KB descriptors for the p = t % 128 layout).
The 0/1 segment-indicator ind[p, q, j] = (pos[j] <= 64p + q < pos[j+1]) is
built in two wide DVE ops. Segment sums accumulate on the TensorEngine as
psum += ind_q.T @ x_q over the 64 q-slices, with even/odd q packed into
separate PE column groups (concurrent sub-array matmuls) to halve the fp32
matmul wall time. Finally y = (psum_even + psum_odd) * 1/max(count, 1).
"""

import os
import sys

import numpy as np

sys.path.insert(0, "/opt/trn_rl_repo")

import concourse.bacc as bacc
import concourse.bass as bass
import concourse.tile as tile
from concourse import mybir
from concourse.bass_utils import run_bass_kernel_spmd

dt = mybir.dt
Alu = mybir.AluOpType

# Problem constants (hardcoded per harness contract).
B, T, C, P = 32, 8192, 256, 65
NSEG = P - 1
NCORES = 8
R = B // NCORES          # batch rows per core
NPART = 128              # SBUF partitions
QTOK = T // NPART        # 64 tokens per partition (contiguous in HBM)

CFG = {
    "chunkq": int(os.environ.get("KB_CHUNKQ", "16")),      # q-slices per x DMA
    "col_pack": os.environ.get("KB_COLPACK", "1") == "1",  # even/odd PE col groups
    "x_bufs": int(os.environ.get("KB_XBUFS", "6")),
    "ind_bufs": int(os.environ.get("KB_INDBUFS", "2")),
    "psum_bufs": int(os.environ.get("KB_PSUMBUFS", "2")),
    "dual_dma": os.environ.get("KB_DUALDMA", "1") == "1",
}


def build_program(cfg=CFG):
    chunkq = cfg["chunkq"]
    nchunk = QTOK // chunkq
    col_pack = cfg["col_pack"]

    nc = bacc.Bacc("TRN2", target_bir_lowering=False, debug=False)

    x_d = nc.dram_tensor("x", [R, T, C], dt.float32, kind="ExternalInput")
    pos_d = nc.dram_tensor("x_pos", [R, P], dt.int32, kind="ExternalInput")
    y_d = nc.dram_tensor("y", [R, NSEG, C], dt.float32, kind="ExternalOutput")

    with tile.TileContext(nc) as tc:
        with (
            tc.tile_pool(name="const", bufs=1) as constp,
            tc.tile_pool(name="xp", bufs=cfg["x_bufs"]) as xp,
            tc.tile_pool(name="sp", bufs=1) as sp,
            tc.tile_pool(name="indp", bufs=cfg["ind_bufs"]) as indp,
            tc.tile_pool(name="smallp", bufs=4) as smallp,
            tc.tile_pool(name="outp", bufs=2) as outp,
            tc.tile_pool(name="psp", bufs=cfg["psum_bufs"], space="PSUM") as psp,
        ):
            # q along the free axis, const across partitions/segments.
            # Values <= 63, exact in f32.
            qio_b = constp.tile([NPART, QTOK, P], dt.float32)
            nc.gpsimd.iota(qio_b[:], pattern=[[1, QTOK], [0, P]], base=0,
                           channel_multiplier=0, allow_small_or_imprecise_dtypes=True)
            # 64*p as a per-partition scalar (<= 8128, exact in f32).
            p64_iota = constp.tile([NPART, 1], dt.float32)
            nc.gpsimd.iota(p64_iota[:], pattern=[[1, 1]], base=0, channel_multiplier=QTOK,
                           allow_small_or_imprecise_dtypes=True)

            for r in range(R):
                # pos row -> [1, 65] f32, broadcast to [128, 65].
                pos_row = smallp.tile([1, P], dt.int32)
                nc.gpsimd.dma_start(pos_row[:], pos_d[r : r + 1, :])
                posf_row = smallp.tile([1, P], dt.float32)
                nc.vector.tensor_copy(posf_row[:], pos_row[:])
                pos_b = smallp.tile([NPART, P], dt.float32)
                nc.gpsimd.partition_broadcast(pos_b[:], posf_row[:])

                # counts -> 1/max(cnt, 1), partition-major [64, 1].
                pos_lo = smallp.tile([NSEG, 1], dt.int32)
                pos_hi = smallp.tile([NSEG, 1], dt.int32)
                nc.gpsimd.dma_start(pos_lo[:], pos_d[r : r + 1, 0:NSEG].rearrange("one p -> p one"))
                nc.gpsimd.dma_start(pos_hi[:], pos_d[r : r + 1, 1:P].rearrange("one p -> p one"))
                cnt_f = smallp.tile([NSEG, 1], dt.float32)
                nc.vector.tensor_tensor(cnt_f[:], pos_hi[:], pos_lo[:], op=Alu.subtract)
                cntc = smallp.tile([NSEG, 1], dt.float32)
                nc.vector.tensor_scalar(cntc[:], cnt_f[:], 1.0, None, op0=Alu.max)
                recip = smallp.tile([NSEG, 1], dt.float32)
                nc.vector.reciprocal(recip[:], cntc[:])

                # S[p, q, j] = (pos[j] <= 64p + q), one fused DVE op.
                S_all = sp.tile([NPART, QTOK, P], dt.float32, tag="sall")
                nc.vector.scalar_tensor_tensor(
                    S_all[:],
                    pos_b[:, None, :].broadcast_to((NPART, QTOK, P)),
                    p64_iota[:],
                    qio_b[:],
                    op0=Alu.subtract,
                    op1=Alu.is_le,
                )
                # ind[p, q, j] = S[p, q, j] - S[p, q, j+1]
                ind_all = indp.tile([NPART, QTOK, NSEG], dt.float32, tag="ind")
                nc.vector.tensor_tensor(
                    ind_all[:], S_all[:, :, 0:NSEG], S_all[:, :, 1:P], op=Alu.subtract
                )

                ps = psp.tile([2 * NSEG if col_pack else NSEG, C], dt.float32)
                # Row as [128 partitions, 64*256]: partition p's line is the
                # contiguous HBM range of tokens [64p, 64p+64).
                xr = x_d[r].rearrange("(p q) c -> p (q c)", p=NPART)
                for ci in range(nchunk):
                    xt = xp.tile([NPART, chunkq * C], dt.float32)
                    eng = nc.scalar if (cfg["dual_dma"] and ci % 2) else nc.sync
                    eng.dma_start(xt[:], xr[:, ci * chunkq * C : (ci + 1) * chunkq * C])
                    for k in range(chunkq):
                        q = ci * chunkq + k
                        rhs = xt[:, k * C : (k + 1) * C]
                        lhsT = ind_all[:, q, :]
                        if col_pack:
                            half = q % 2
                            nc.tensor.matmul(
                                ps[half * NSEG : (half + 1) * NSEG, :], lhsT, rhs,
                                start=(q == half), stop=(q == QTOK - 2 + half),
                                tile_position=(0, half * NSEG),
                                skip_group_check=True,
                            )
                        else:
                            nc.tensor.matmul(
                                ps[:], lhsT, rhs,
                                start=(q == 0), stop=(q == QTOK - 1),
                            )

                out_t = outp.tile([NSEG, C], dt.float32)
                if col_pack:
                    # DVE reads one PSUM operand per op: scale each half alone.
                    half_t = outp.tile([NSEG, C], dt.float32, tag="half")
                    nc.vector.tensor_scalar(
                        half_t[:], ps[NSEG : 2 * NSEG, :], recip[:], None, op0=Alu.mult
                    )
                    nc.vector.scalar_tensor_tensor(
                        out_t[:], ps[0:NSEG, :], recip[:], half_t[:],
                        op0=Alu.mult, op1=Alu.add,
                    )
                else:
                    nc.vector.tensor_scalar(out_t[:], ps[:], recip[:], None, op0=Alu.mult)
                nc.gpsimd.dma_start(y_d[r], out_t[:])

    nc.compile()
    return nc


_PROGRAM = None


def _get_program():
    global _PROGRAM
    if _PROGRAM is None:
        _PROGRAM = build_program()
    return _PROGRAM


def kernel(x, x_pos):
    x = np.ascontiguousarray(x, dtype=np.float32)
    x_pos = np.ascontiguousarray(x_pos, dtype=np.int32)
    nc = _get_program()
    in_maps = [
        {"x": x[c * R : (c + 1) * R], "x_pos": x_pos[c * R : (c + 1) * R]}
        for c in range(NCORES)
    ]
    res = run_bass_kernel_spmd(nc, in_maps, list(range(NCORES)))
    y = np.concatenate([res.results[c]["y"] for c in range(NCORES)], axis=0)
    return y.astype(np.float32)


# revision 3
# speedup vs baseline: 1.1286x; 1.1026x over previous
"""Segment mean-pool (LocalPooling1D) Trainium2 Bass kernel.

x [32, 8192, 256] f32, x_pos [32, 65] sorted int32 boundaries -> y [32, 64, 256].
y[b, j] = mean(x[b, x_pos[b,j]:x_pos[b,j+1]]), empty segments -> 0.

Strategy: data-parallel over batch, 4 rows per core on 8 cores. Token t of a
row maps to SBUF partition p = t // 64, free-slot q = t % 64, so every
partition's x data is one contiguous 64 KB HBM chunk -> large DMA descriptors
at near-peak HBM bandwidth. The 0/1 segment-indicator
ind[p, q, j] = (pos[j] <= 64p + q < pos[j+1]) is built on the DVE per x-chunk
(so the first matmul can start a few us in, not after a whole-row build), from
a tiny [128, QTOK] q-iota and a [128, P] broadcast of pos, both double
stride-0-broadcast into the fused compare. Segment sums accumulate on the
TensorEngine as psum += ind_q.T @ x_q over the 64 q-slices, with even/odd q
packed into separate PE column groups (concurrent sub-array matmuls). Finally
y = (psum_even + psum_odd) * 1/max(count, 1).
"""

import os
import sys

import numpy as np

sys.path.insert(0, "/opt/trn_rl_repo")

import concourse.bacc as bacc
import concourse.bass as bass
import concourse.tile as tile
from concourse import mybir
from concourse.bass_utils import run_bass_kernel_spmd

dt = mybir.dt
Alu = mybir.AluOpType

# Problem constants (hardcoded per harness contract).
B, T, C, P = 32, 8192, 256, 65
NSEG = P - 1
NCORES = 8
R = B // NCORES          # batch rows per core
NPART = 128              # SBUF partitions
QTOK = T // NPART        # 64 tokens per partition (contiguous in HBM)

CFG = {
    "chunkq": int(os.environ.get("KB_CHUNKQ", "16")),      # q-slices per x DMA
    "col_pack": os.environ.get("KB_COLPACK", "1") == "1",  # even/odd PE col groups
    "x_bufs": int(os.environ.get("KB_XBUFS", "6")),
    "ind_bufs": int(os.environ.get("KB_INDBUFS", "6")),
    "s_bufs": int(os.environ.get("KB_SBUFS", "3")),
    "psum_bufs": int(os.environ.get("KB_PSUMBUFS", "2")),
    "dual_dma": os.environ.get("KB_DUALDMA", "1") == "1",
}


def build_program(cfg=CFG):
    chunkq = cfg["chunkq"]
    nchunk = QTOK // chunkq
    col_pack = cfg["col_pack"]

    nc = bacc.Bacc("TRN2", target_bir_lowering=False, debug=False)

    x_d = nc.dram_tensor("x", [R, T, C], dt.float32, kind="ExternalInput")
    pos_d = nc.dram_tensor("x_pos", [R, P], dt.int32, kind="ExternalInput")
    y_d = nc.dram_tensor("y", [R, NSEG, C], dt.float32, kind="ExternalOutput")

    with tile.TileContext(nc) as tc:
        with (
            tc.tile_pool(name="const", bufs=1) as constp,
            tc.tile_pool(name="xp", bufs=cfg["x_bufs"]) as xp,
            tc.tile_pool(name="sp", bufs=cfg["s_bufs"]) as sp,
            tc.tile_pool(name="indp", bufs=cfg["ind_bufs"]) as indp,
            tc.tile_pool(name="smallp", bufs=R) as smallp,
            tc.tile_pool(name="outp", bufs=2) as outp,
            tc.tile_pool(name="psp", bufs=cfg["psum_bufs"], space="PSUM") as psp,
        ):
            # q (token index within partition) along the free axis: [128, 64].
            q_sm = constp.tile([NPART, QTOK], dt.float32)
            nc.gpsimd.iota(q_sm[:], pattern=[[1, QTOK]], base=0,
                           channel_multiplier=0, allow_small_or_imprecise_dtypes=True)
            # 64*p as a per-partition scalar (<= 8128, exact in f32).
            p64_iota = constp.tile([NPART, 1], dt.float32)
            nc.gpsimd.iota(p64_iota[:], pattern=[[1, 1]], base=0, channel_multiplier=QTOK,
                           allow_small_or_imprecise_dtypes=True)

            # Critical startup chain first: pos rows -> f32 -> [128, P]
            # broadcasts, so row 0's indicator can build within a few us.
            pos_rows, posf_rows, pos_bs = [], [], []
            for r in range(R):
                pos_row = smallp.tile([1, P], dt.int32, tag="posrow")
                nc.gpsimd.dma_start(pos_row[:], pos_d[r : r + 1, :])
                pos_rows.append(pos_row)
            for r in range(R):
                posf_row = smallp.tile([1, P], dt.float32, tag="posf")
                nc.vector.tensor_copy(posf_row[:], pos_rows[r][:])
                posf_rows.append(posf_row)
            for r in range(R):
                pos_b = smallp.tile([NPART, P], dt.float32, tag="posb")
                nc.gpsimd.partition_broadcast(pos_b[:], posf_rows[r][:])
                pos_bs.append(pos_b)

            # Off the critical path: segment counts -> 1/max(cnt, 1) per row.
            pos_los, pos_his = [], []
            for r in range(R):
                pos_lo = smallp.tile([NSEG, 1], dt.int32, tag="poslo")
                pos_hi = smallp.tile([NSEG, 1], dt.int32, tag="poshi")
                nc.gpsimd.dma_start(pos_lo[:], pos_d[r : r + 1, 0:NSEG].rearrange("one p -> p one"))
                nc.gpsimd.dma_start(pos_hi[:], pos_d[r : r + 1, 1:P].rearrange("one p -> p one"))
                pos_los.append(pos_lo)
                pos_his.append(pos_hi)

            for r in range(R):
                pos_b = pos_bs[r]

                ps = psp.tile([2 * NSEG if col_pack else NSEG, C], dt.float32)
                # Row as [128 partitions, 64*256]: partition p's line is the
                # contiguous HBM range of tokens [64p, 64p+64).
                xr = x_d[r].rearrange("(p q) c -> p (q c)", p=NPART)
                for ci in range(nchunk):
                    # S[p, k, j] = (pos[j] <= 64p + q), q = ci*chunkq + k.
                    S_c = sp.tile([NPART, chunkq, P], dt.float32, tag="sall")
                    nc.vector.scalar_tensor_tensor(
                        S_c[:],
                        pos_b[:, None, :].broadcast_to((NPART, chunkq, P)),
                        p64_iota[:],
                        q_sm[:, ci * chunkq : (ci + 1) * chunkq, None].broadcast_to(
                            (NPART, chunkq, P)
                        ),
                        op0=Alu.subtract,
                        op1=Alu.is_le,
                    )
                    # ind[p, k, j] = S[p, k, j] - S[p, k, j+1]
                    ind_c = indp.tile([NPART, chunkq, NSEG], dt.float32, tag="ind")
                    nc.vector.tensor_tensor(
                        ind_c[:], S_c[:, :, 0:NSEG], S_c[:, :, 1:P], op=Alu.subtract
                    )

                    xt = xp.tile([NPART, chunkq * C], dt.float32)
                    eng = nc.scalar if (cfg["dual_dma"] and ci % 2) else nc.sync
                    eng.dma_start(xt[:], xr[:, ci * chunkq * C : (ci + 1) * chunkq * C])
                    for k in range(chunkq):
                        q = ci * chunkq + k
                        rhs = xt[:, k * C : (k + 1) * C]
                        lhsT = ind_c[:, k, :]
                        if col_pack:
                            half = q % 2
                            nc.tensor.matmul(
                                ps[half * NSEG : (half + 1) * NSEG, :], lhsT, rhs,
                                start=(q == half), stop=(q == QTOK - 2 + half),
                                tile_position=(0, half * NSEG),
                                skip_group_check=True,
                            )
                        else:
                            nc.tensor.matmul(
                                ps[:], lhsT, rhs,
                                start=(q == 0), stop=(q == QTOK - 1),
                            )

                # Segment counts (off matmul critical path, before the scale).
                cnt_f = smallp.tile([NSEG, 1], dt.float32, tag="cnt")
                nc.vector.tensor_tensor(cnt_f[:], pos_his[r][:], pos_los[r][:], op=Alu.subtract)
                cntc = smallp.tile([NSEG, 1], dt.float32, tag="cntc")
                nc.vector.tensor_scalar(cntc[:], cnt_f[:], 1.0, None, op0=Alu.max)
                recip = smallp.tile([NSEG, 1], dt.float32, tag="recip")
                nc.vector.reciprocal(recip[:], cntc[:])

                out_t = outp.tile([NSEG, C], dt.float32)
                if col_pack:
                    # DVE reads one PSUM operand per op: scale each half alone.
                    half_t = outp.tile([NSEG, C], dt.float32, tag="half")
                    nc.vector.tensor_scalar(
                        half_t[:], ps[NSEG : 2 * NSEG, :], recip[:], None, op0=Alu.mult
                    )
                    nc.vector.scalar_tensor_tensor(
                        out_t[:], ps[0:NSEG, :], recip[:], half_t[:],
                        op0=Alu.mult, op1=Alu.add,
                    )
                else:
                    nc.vector.tensor_scalar(out_t[:], ps[:], recip[:], None, op0=Alu.mult)
                nc.gpsimd.dma_start(y_d[r], out_t[:])

    nc.compile()
    return nc


_PROGRAM = None


def _get_program():
    global _PROGRAM
    if _PROGRAM is None:
        _PROGRAM = build_program()
    return _PROGRAM


def kernel(x, x_pos):
    x = np.ascontiguousarray(x, dtype=np.float32)
    x_pos = np.ascontiguousarray(x_pos, dtype=np.int32)
    nc = _get_program()
    in_maps = [
        {"x": x[c * R : (c + 1) * R], "x_pos": x_pos[c * R : (c + 1) * R]}
        for c in range(NCORES)
    ]
    res = run_bass_kernel_spmd(nc, in_maps, list(range(NCORES)))
    y = np.concatenate([res.results[c]["y"] for c in range(NCORES)], axis=0)
    return y.astype(np.float32)
